# revision 1
# baseline (speedup 1.0000x reference)
"""Trainium2 Bass kernel for ActivationSparseLinear (batched GEMV).

out[b, 0, n] = sum_k x[b, 0, k] * weight[n, k]
  x: (8, 1, 4096) f32, weight: (11008, 4096) f32 -> out: (8, 1, 11008) f32

Strategy (tensor-parallel over out_features, 8 NeuronCores):
  - Each core owns 1376 rows of `weight` and the full (tiny) `x`.
  - Memory-bound on the f32 weight stream (~22.5 MB/core).  The weight is
    DMA'd with an on-the-fly f32->bf16 cast (SWDGE), PE-transposed in
    128x128 bf16 tiles (k onto partitions), copied PSUM->SBUF on
    DVE/ACT, then consumed by bf16 matmuls (lhsT = W^T tile,
    rhs = host-pretransposed x^T) accumulating f32 in PSUM.
  - No cross-core communication; the host concatenates the 8 output shards.
"""

from contextlib import ExitStack

import numpy as np

import concourse.bass as bass
import concourse.bacc as bacc
import concourse.mybir as mybir
import concourse.tile as tile
from concourse.bass_utils import run_bass_kernel_spmd

B = 8          # batch (seq_len 1 folded away)
K = 4096       # in_features
N = 11008      # out_features
NCORES = 8
N_SHARD = N // NCORES          # 1376 rows per core
KT = K // 128                  # 32 k-tiles
NT = (N_SHARD + 127) // 128    # 11 n-tiles (last has 96 rows)
CHUNK = 8                      # k-tiles per PSUM transpose chunk (one bank)

_GRAPH_CACHE = {}


def build_graph() -> bacc.Bacc:
    nc = bacc.Bacc("TRN2", target_bir_lowering=False, debug=False,
                   num_devices=NCORES)
    w = nc.declare_dram_parameter("w", [N_SHARD, K], mybir.dt.float32,
                                  isOutput=False)
    xt = nc.declare_dram_parameter("xt", [128, KT * B], mybir.dt.float32,
                                   isOutput=False)
    ident = nc.declare_dram_parameter("ident", [128, 128], mybir.dt.float32,
                                      isOutput=False)
    out = nc.declare_dram_parameter("out", [N_SHARD, B], mybir.dt.float32,
                                    isOutput=True)

    bf16 = mybir.dt.bfloat16
    f32 = mybir.dt.float32

    with tile.TileContext(nc) as tc, ExitStack() as ctx:
        const_pool = ctx.enter_context(tc.tile_pool(name="const", bufs=1))
        wn_pool = ctx.enter_context(tc.tile_pool(name="wn", bufs=2))
        wt_pool = ctx.enter_context(tc.tile_pool(name="wt", bufs=4))
        pst_pool = ctx.enter_context(
            tc.tile_pool(name="pst", bufs=4, space="PSUM"))
        psa_pool = ctx.enter_context(
            tc.tile_pool(name="psa", bufs=2, space="PSUM"))
        out_pool = ctx.enter_context(tc.tile_pool(name="outp", bufs=2))

        # constants: x^T (pre-transposed on host to [k_in_tile, kt*B]) and
        # the transpose identity, both cast f32->bf16 during the DMA.
        xt_sb = const_pool.tile([128, KT * B], bf16)
        nc.gpsimd.dma_start(xt_sb[:], xt[:])
        id_sb = const_pool.tile([128, 128], bf16)
        nc.gpsimd.dma_start(id_sb[:], ident[:])

        n_copy = 0
        for nt in range(NT):
            rows = min(128, N_SHARD - nt * 128)
            w_sb = wn_pool.tile([128, K], bf16, tag="w_sb")
            # one 2 MB (f32-side) cast-DMA per 128 weight rows
            nc.gpsimd.dma_start(w_sb[:rows, :], w[nt * 128:nt * 128 + rows, :])

            acc_ps = psa_pool.tile([128, B], f32, tag="acc")
            for c in range(KT // CHUNK):
                tp_ps = pst_pool.tile([128, CHUNK * 128], bf16, tag="tp")
                wt_sb = wt_pool.tile([128, CHUNK * 128], bf16, tag="wt")
                for j in range(CHUNK):
                    kt = c * CHUNK + j
                    nc.tensor.transpose(
                        tp_ps[:, j * 128:j * 128 + rows],
                        w_sb[:rows, kt * 128:(kt + 1) * 128],
                        id_sb[:rows, :rows],
                    )
                # PSUM -> SBUF bounce, alternating DVE / ACT
                if n_copy % 2 == 0:
                    nc.vector.tensor_copy(wt_sb[:], tp_ps[:])
                else:
                    nc.scalar.copy(wt_sb[:], tp_ps[:])
                n_copy += 1
                for j in range(CHUNK):
                    kt = c * CHUNK + j
                    nc.tensor.matmul(
                        acc_ps[:rows, :],
                        wt_sb[:, j * 128:j * 128 + rows],
                        xt_sb[:, kt * B:(kt + 1) * B],
                        start=(kt == 0),
                        stop=(kt == KT - 1),
                    )
            o_sb = out_pool.tile([128, B], f32, tag="o")
            nc.scalar.copy(o_sb[:rows, :], acc_ps[:rows, :])
            nc.sync.dma_start(out[nt * 128:nt * 128 + rows, :], o_sb[:rows, :])

    nc.compile()
    return nc


def _get_graph() -> bacc.Bacc:
    if "nc" not in _GRAPH_CACHE:
        _GRAPH_CACHE["nc"] = build_graph()
    return _GRAPH_CACHE["nc"]


def _make_in_maps(x: np.ndarray, weight: np.ndarray):
    x = np.asarray(x, dtype=np.float32).reshape(B, K)
    weight = np.asarray(weight, dtype=np.float32)
    # xt[p, kt*B + b] = x[b, kt*128 + p]
    xt = np.ascontiguousarray(
        x.reshape(B, KT, 128).transpose(2, 1, 0).reshape(128, KT * B))
    ident = np.eye(128, dtype=np.float32)
    in_maps = []
    for c in range(NCORES):
        w_shard = np.ascontiguousarray(
            weight[c * N_SHARD:(c + 1) * N_SHARD, :])
        in_maps.append({"w": w_shard, "xt": xt, "ident": ident})
    return in_maps


def _run(x: np.ndarray, weight: np.ndarray, trace: bool = False):
    nc = _get_graph()
    in_maps = _make_in_maps(x, weight)
    res = run_bass_kernel_spmd(nc, in_maps, core_ids=list(range(NCORES)),
                               trace=trace)
    out = np.empty((B, 1, N), dtype=np.float32)
    for c in range(NCORES):
        out[:, 0, c * N_SHARD:(c + 1) * N_SHARD] = res.results[c]["out"].T
    return out, res


def kernel(x: np.ndarray, weight: np.ndarray) -> np.ndarray:
    out, _ = _run(x, weight, trace=False)
    return out


# revision 2
# speedup vs baseline: 1.0528x; 1.0528x over previous
"""Trainium2 Bass kernel for ActivationSparseLinear (batched GEMV).

out[b, 0, n] = sum_k x[b, 0, k] * weight[n, k]
  x: (8, 1, 4096) f32, weight: (11008, 4096) f32 -> out: (8, 1, 11008) f32

Strategy (tensor-parallel over out_features, 8 NeuronCores):
  - Each core owns 1376 rows of `weight` and the full (tiny) `x`.
  - Memory-bound on the f32 weight stream (~22.5 MB/core).  The weight is
    DMA'd with an on-the-fly f32->bf16 cast (SWDGE), PE-transposed in
    128x128 bf16 tiles (k onto partitions), bounced PSUM->SBUF on
    DVE/ACT, then consumed as the 512-wide MOVING operand of bf16
    matmuls whose stationary operand is the 8-column x^T tile
    (cheap LDWEIGHTS), accumulating f32 [8, 512] in PSUM.
  - No cross-core communication; the host concatenates the 8 output shards.
"""

from contextlib import ExitStack

import numpy as np

import concourse.bass as bass
import concourse.bacc as bacc
import concourse.mybir as mybir
import concourse.tile as tile
from concourse.bass_utils import run_bass_kernel_spmd

B = 8          # batch (seq_len 1 folded away)
K = 4096       # in_features
N = 11008      # out_features
NCORES = 8
N_SHARD = N // NCORES          # 1376 rows per core
KT = K // 128                  # 32 k-tiles
NCHUNK = 512                   # output rows per psum accumulator chunk
KSEG = 1024                    # k columns per weight DMA segment (2MB f32)

_GRAPH_CACHE = {}


def build_graph() -> bacc.Bacc:
    nc = bacc.Bacc("TRN2", target_bir_lowering=False, debug=False,
                   num_devices=NCORES)
    w = nc.declare_dram_parameter("w", [N_SHARD, K], mybir.dt.float32,
                                  isOutput=False)
    xt = nc.declare_dram_parameter("xt", [128, KT * B], mybir.dt.float32,
                                   isOutput=False)
    ident = nc.declare_dram_parameter("ident", [128, 128], mybir.dt.float32,
                                      isOutput=False)
    out = nc.declare_dram_parameter("out", [B, N_SHARD], mybir.dt.float32,
                                    isOutput=True)

    bf16 = mybir.dt.bfloat16
    f32 = mybir.dt.float32

    chunks = []  # (row0, nrows) output chunks of <=512 rows
    r = 0
    while r < N_SHARD:
        chunks.append((r, min(NCHUNK, N_SHARD - r)))
        r += NCHUNK

    with tile.TileContext(nc) as tc, ExitStack() as ctx:
        const_pool = ctx.enter_context(tc.tile_pool(name="const", bufs=1))
        wn_pool = ctx.enter_context(tc.tile_pool(name="wn", bufs=6))
        wt_pool = ctx.enter_context(tc.tile_pool(name="wt", bufs=6))
        pst_pool = ctx.enter_context(
            tc.tile_pool(name="pst", bufs=4, space="PSUM"))
        psa_pool = ctx.enter_context(
            tc.tile_pool(name="psa", bufs=2, space="PSUM"))
        out_pool = ctx.enter_context(tc.tile_pool(name="outp", bufs=2))

        # constants: x^T (host-pretransposed to [k_in_tile, kt*B]) and the
        # transpose identity, both cast f32->bf16 during the DMA.
        xt_sb = const_pool.tile([128, KT * B], bf16)
        nc.gpsimd.dma_start(xt_sb[:], xt[:])
        id_sb = const_pool.tile([128, 128], bf16)
        nc.gpsimd.dma_start(id_sb[:], ident[:])

        n_copy = 0
        for row0, nrows in chunks:
            # number of 128-row n-tiles in this chunk (last may be 96 rows)
            jtiles = [(j, min(128, nrows - j * 128))
                      for j in range((nrows + 127) // 128)]
            acc_ps = psa_pool.tile([B, NCHUNK], f32, tag="acc")

            # segment tiles: w_seg[p, j, kk] = w[row0 + j*128 + p, s*KSEG + kk]
            segs = []
            for s in range(K // KSEG):
                w_sb = wn_pool.tile([128, len(jtiles), KSEG], bf16, tag="w_sb")
                src = w[row0:row0 + nrows, s * KSEG:(s + 1) * KSEG]
                src = src.rearrange("(j p) k -> p j k", p=128) \
                    if nrows % 128 == 0 else None
                if src is None:
                    # tail chunk: last j-tile has 96 rows; issue per-j DMAs
                    for j, jr in jtiles:
                        nc.gpsimd.dma_start(
                            w_sb[:jr, j, :],
                            w[row0 + j * 128:row0 + j * 128 + jr,
                              s * KSEG:(s + 1) * KSEG])
                else:
                    nc.gpsimd.dma_start(w_sb[:], src)
                segs.append(w_sb)

            for s, w_sb in enumerate(segs):
                for kk in range(KSEG // 128):
                    kt = s * (KSEG // 128) + kk
                    tp_ps = pst_pool.tile([128, NCHUNK], bf16, tag="tp")
                    wt_sb = wt_pool.tile([128, NCHUNK], bf16, tag="wt")
                    for j, jr in jtiles:
                        nc.tensor.transpose(
                            tp_ps[:, j * 128:j * 128 + jr],
                            w_sb[:jr, j, kk * 128:(kk + 1) * 128],
                            id_sb[:jr, :jr],
                        )
                    if n_copy % 2 == 0:
                        nc.vector.tensor_copy(wt_sb[:, :nrows],
                                              tp_ps[:, :nrows])
                    else:
                        nc.scalar.copy(wt_sb[:, :nrows], tp_ps[:, :nrows])
                    n_copy += 1
                    nc.tensor.matmul(
                        acc_ps[:, :nrows],
                        xt_sb[:, kt * B:(kt + 1) * B],
                        wt_sb[:, :nrows],
                        start=(kt == 0),
                        stop=(kt == KT - 1),
                    )
            o_sb = out_pool.tile([B, NCHUNK], f32, tag="o")
            nc.scalar.copy(o_sb[:, :nrows], acc_ps[:, :nrows])
            nc.sync.dma_start(out[:, row0:row0 + nrows], o_sb[:, :nrows])

    nc.compile()
    return nc


def _get_graph() -> bacc.Bacc:
    if "nc" not in _GRAPH_CACHE:
        _GRAPH_CACHE["nc"] = build_graph()
    return _GRAPH_CACHE["nc"]


def _make_in_maps(x: np.ndarray, weight: np.ndarray):
    x = np.asarray(x, dtype=np.float32).reshape(B, K)
    weight = np.asarray(weight, dtype=np.float32)
    # xt[p, kt*B + b] = x[b, kt*128 + p]
    xt = np.ascontiguousarray(
        x.reshape(B, KT, 128).transpose(2, 1, 0).reshape(128, KT * B))
    ident = np.eye(128, dtype=np.float32)
    in_maps = []
    for c in range(NCORES):
        w_shard = np.ascontiguousarray(
            weight[c * N_SHARD:(c + 1) * N_SHARD, :])
        in_maps.append({"w": w_shard, "xt": xt, "ident": ident})
    return in_maps


def _run(x: np.ndarray, weight: np.ndarray, trace: bool = False):
    nc = _get_graph()
    in_maps = _make_in_maps(x, weight)
    res = run_bass_kernel_spmd(nc, in_maps, core_ids=list(range(NCORES)),
                               trace=trace)
    out = np.empty((B, 1, N), dtype=np.float32)
    for c in range(NCORES):
        out[:, 0, c * N_SHARD:(c + 1) * N_SHARD] = res.results[c]["out"]
    return out, res


def kernel(x: np.ndarray, weight: np.ndarray) -> np.ndarray:
    out, _ = _run(x, weight, trace=False)
    return out


# revision 6
# speedup vs baseline: 1.1133x; 1.0574x over previous
"""Trainium2 Bass kernel for ActivationSparseLinear (batched GEMV).

out[b, 0, n] = sum_k x[b, 0, k] * weight[n, k]
  x: (8, 1, 4096) f32, weight: (11008, 4096) f32 -> out: (8, 1, 11008) f32

Strategy (tensor-parallel over out_features, 8 NeuronCores):
  - Each core owns 1376 rows of `weight` and the full (tiny) `x`.
  - Memory-bound on the f32 weight stream (~22.5 MB/core).  The weight is
    DMA'd with an on-the-fly f32->bf16 cast (SWDGE), PE-transposed in
    128x128 bf16 tiles (k onto partitions), bounced PSUM->SBUF on
    DVE/ACT, then consumed as the 512-wide MOVING operand of bf16
    matmuls whose stationary operand is the 8-column x^T tile
    (cheap LDWEIGHTS), accumulating f32 [8, 512] in PSUM.
  - No cross-core communication; the host concatenates the 8 output shards.
"""

from contextlib import ExitStack

import numpy as np

import concourse.bass as bass
import concourse.bacc as bacc
import concourse.mybir as mybir
import concourse.tile as tile
from concourse.bass_utils import run_bass_kernel_spmd

B = 8          # batch (seq_len 1 folded away)
K = 4096       # in_features
N = 11008      # out_features
NCORES = 8
N_SHARD = N // NCORES          # 1376 rows per core
KT = K // 128                  # 32 k-tiles
NCHUNK = 512                   # output rows per psum accumulator chunk
KSEG = 1024                    # k columns per weight DMA segment (2MB f32)

_GRAPH_CACHE = {}


def build_graph() -> bacc.Bacc:
    nc = bacc.Bacc("TRN2", target_bir_lowering=False, debug=False,
                   num_devices=NCORES)
    w = nc.declare_dram_parameter("w", [N_SHARD, K], mybir.dt.float32,
                                  isOutput=False)
    xt = nc.declare_dram_parameter("xt", [128, KT * B], mybir.dt.bfloat16,
                                   isOutput=False)
    ident = nc.declare_dram_parameter("ident", [128, 128], mybir.dt.bfloat16,
                                      isOutput=False)
    out = nc.declare_dram_parameter("out", [B, N_SHARD], mybir.dt.float32,
                                    isOutput=True)

    bf16 = mybir.dt.bfloat16
    f32 = mybir.dt.float32

    chunks = []  # (row0, nrows) output chunks of <=512 rows
    r = 0
    while r < N_SHARD:
        chunks.append((r, min(NCHUNK, N_SHARD - r)))
        r += NCHUNK

    with tile.TileContext(nc) as tc, ExitStack() as ctx:
        const_pool = ctx.enter_context(tc.tile_pool(name="const", bufs=1))
        wn_pool = ctx.enter_context(tc.tile_pool(name="wn", bufs=6))
        wt_pool = ctx.enter_context(tc.tile_pool(name="wt", bufs=6))
        pst_pool = ctx.enter_context(
            tc.tile_pool(name="pst", bufs=4, space="PSUM"))
        psa_pool = ctx.enter_context(
            tc.tile_pool(name="psa", bufs=2, space="PSUM"))
        out_pool = ctx.enter_context(tc.tile_pool(name="outp", bufs=2))

        # constants: x^T (host-pretransposed to [k_in_tile, kt*B]) and the
        # transpose identity, already bf16 on host; HWDGE load keeps the
        # gpsimd SWDGE queue free for the weight stream.
        xt_sb = const_pool.tile([128, KT * B], bf16)
        nc.sync.dma_start(xt_sb[:], xt[:])
        id_sb = const_pool.tile([128, 128], bf16)
        nc.sync.dma_start(id_sb[:], ident[:])

        n_copy = 0
        for row0, nrows in chunks:
            # number of 128-row n-tiles in this chunk (last may be 96 rows)
            jtiles = [(j, min(128, nrows - j * 128))
                      for j in range((nrows + 127) // 128)]
            acc_ps = psa_pool.tile([B, NCHUNK], f32, tag="acc")

            # segment tiles: w_seg[p, j, kk] = w[row0 + j*128 + p, s*KSEG + kk]
            segs = []
            for s in range(K // KSEG):
                w_sb = wn_pool.tile([128, len(jtiles), KSEG], bf16, tag="w_sb")
                src = w[row0:row0 + nrows, s * KSEG:(s + 1) * KSEG]
                src = src.rearrange("(j p) k -> p j k", p=128) \
                    if nrows % 128 == 0 else None
                if src is None:
                    # tail chunk: last j-tile has 96 rows; issue per-j DMAs
                    for j, jr in jtiles:
                        nc.gpsimd.dma_start(
                            w_sb[:jr, j, :],
                            w[row0 + j * 128:row0 + j * 128 + jr,
                              s * KSEG:(s + 1) * KSEG])
                else:
                    nc.gpsimd.dma_start(w_sb[:], src)
                segs.append(w_sb)

            for s, w_sb in enumerate(segs):
                for kk in range(KSEG // 128):
                    kt = s * (KSEG // 128) + kk
                    tp_ps = pst_pool.tile([128, NCHUNK], f32, tag="tp")
                    wt_sb = wt_pool.tile([128, NCHUNK], bf16, tag="wt")
                    for j, jr in jtiles:
                        # transpose as a NORMAL matmul with identity rhs
                        # (out = w_tile.T @ I) so the PE's HAM clock-gate
                        # sees real matmul activity and un-throttles;
                        # bf16 weights also get the fast weight load.
                        nc.tensor.matmul(
                            tp_ps[:, j * 128:j * 128 + jr],
                            w_sb[:jr, j, kk * 128:(kk + 1) * 128],
                            id_sb[:jr, :jr],
                        )
                    # copy casts the exact f32 psum values back to bf16;
                    # DVE gets ~8/13 of copies, ACT ~5/13 (measured rates)
                    if (n_copy * 8) % 13 < 8:
                        nc.vector.tensor_copy(wt_sb[:, :nrows],
                                              tp_ps[:, :nrows])
                    else:
                        nc.scalar.copy(wt_sb[:, :nrows], tp_ps[:, :nrows])
                    n_copy += 1
                    nc.tensor.matmul(
                        acc_ps[:, :nrows],
                        xt_sb[:, kt * B:(kt + 1) * B],
                        wt_sb[:, :nrows],
                        start=(kt == 0),
                        stop=(kt == KT - 1),
                    )
            o_sb = out_pool.tile([B, NCHUNK], f32, tag="o")
            nc.scalar.copy(o_sb[:, :nrows], acc_ps[:, :nrows])
            nc.sync.dma_start(out[:, row0:row0 + nrows], o_sb[:, :nrows])

    nc.compile()
    return nc


def _get_graph() -> bacc.Bacc:
    if "nc" not in _GRAPH_CACHE:
        _GRAPH_CACHE["nc"] = build_graph()
    return _GRAPH_CACHE["nc"]


def _make_in_maps(x: np.ndarray, weight: np.ndarray):
    x = np.asarray(x, dtype=np.float32).reshape(B, K)
    weight = np.asarray(weight, dtype=np.float32)
    bf16_np = mybir.dt.np(mybir.dt.bfloat16)
    # xt[p, kt*B + b] = x[b, kt*128 + p]
    xt = np.ascontiguousarray(
        x.reshape(B, KT, 128).transpose(2, 1, 0).reshape(128, KT * B)
    ).astype(bf16_np)
    ident = np.eye(128, dtype=np.float32).astype(bf16_np)
    in_maps = []
    for c in range(NCORES):
        w_shard = np.ascontiguousarray(
            weight[c * N_SHARD:(c + 1) * N_SHARD, :])
        in_maps.append({"w": w_shard, "xt": xt, "ident": ident})
    return in_maps


def _run(x: np.ndarray, weight: np.ndarray, trace: bool = False):
    nc = _get_graph()
    in_maps = _make_in_maps(x, weight)
    res = run_bass_kernel_spmd(nc, in_maps, core_ids=list(range(NCORES)),
                               trace=trace)
    out = np.empty((B, 1, N), dtype=np.float32)
    for c in range(NCORES):
        out[:, 0, c * N_SHARD:(c + 1) * N_SHARD] = res.results[c]["out"]
    return out, res


def kernel(x: np.ndarray, weight: np.ndarray) -> np.ndarray:
    out, _ = _run(x, weight, trace=False)
    return out
